# revision 8
# baseline (speedup 1.0000x reference)
"""DDSL polygon NUFT kernel for Trainium2 (8 NeuronCores) — freq-major v2.

Math (ring polygon, vertices v, edges e=(v, v+1 mod NV), omega = 2*pi*k):
    phi_v = kx*Vx_v + ky*Vy_v        (unitless phase / 2pi)
    delta = phi - round(phi)         (exact range reduction)
    sh = sin(pi*delta), ch = cos(pi*delta)
    F_re[f] = sum_v A_v * 2*sh^2,  F_im[f] = sum_v A_v * 2*sh*ch
    A_v = (W_{v-1}/rho_{v-1} - W_v/rho_v) / phi_v
    rho_e = phi_e - phi_{e+1},  W_e = -RES^2 * CD_e / (4*pi^2)
(the 2x is folded into W; device W additionally scaled by 2^-S_SCALE for f16
range, host multiplies F back by 2^S_SCALE.)

Device layout: freq-major. A "unit" = (128-frequency block, batch): 128
partitions = frequencies, free dim = 512 ring-ordered vertices. Per unit:
  PE:  phi  = mm(kxy, Vhi) + mm(kxy, Vlo)   (f32r split -> fp32-grade phi)
       rho' = mm(kxy, Vdiff/W)              (f32r, low precision is fine)
       delta: accumulate mm(-I, k) into the phi PSUM bank after phi is read
  ACT: mphi = Copy(phi + MAGIC); k = Copy(mphi - MAGIC) -> bf16 (exact round)
       sh = Sin(delta*pi) -> f16 ; ch = Sin(delta*pi + pi/2) -> f16
  DVE: rphi = 1/phi -> f16 ; rrw = 1/rho' -> f16 (wrap col duplicated)
       ash = A*sh ; t = [ash*sh | ash*ch] (f16, 2x mode)
       F cols via tensor_scalar accum reduction (4x mode, fp32 accum)
  Pool (gpsimd): a' = rrw_{v-1} - rrw_v ; A = a' * rphi   (f16)
Units run in pairs (both batches of a freq block; 1024-col instructions) plus
one single-unit tail. 65 freq blocks x 2 batches = 130 units over 8 cores:
cores 0-5: 8 pairs (+ dummy tail), core 6: 8 pairs + (block 64, b0), core 7:
8 pairs + (block 64, b1). The SPMD tail reads per-core tail-batch data.

Host: ring permutation, weight prep, gather partial F, *2^S_SCALE, DC bin,
exact recompute of (rare) non-finite bins.
"""

import numpy as np

B = 2
NV = 512
RES = 128
FX, FY = 128, 65
NF = FX * FY                 # 8320
N_CORES = 8
FPB = 128                    # freqs per block
N_BLOCKS = NF // FPB         # 65
S_SCALE = 4                  # device weights scaled by 2^-4
T_DVE = ()                   # blocks whose t-product runs on DVE
T_SPLIT = (7,)               # blocks whose t-product splits into re/im halves
PE_WARM = 3                  # dummy matmuls to ramp the PE p-state
FILLER = 0                   # PE keepalive matmuls per block
SPLIT_LAST_DMA = False       # flush block 4-7 cols before the tail
FIRST_SPLIT = False          # split block 0 into two single-batch emits
WBUFS = 6                    # work pool buffers
TAIL_POS = None              # position of the tail emit in the plan
PHIBUFS = 3
RHOBUFS = 1
TAIL_DVE = True              # route the tail block's chain through DVE
AP_DVE = False               # a'-subtract on DVE instead of Pool
R_ACT = ((0, 8), (1, 8))     # (half, block) reduces on ACT
ASH_POOL = ()                # blocks whose ash-product runs on Pool

_MAGIC = float(np.float32(1.5 * 2**23))
_PI = float(np.float32(np.pi))
_HALF_PI = float(np.float32(np.pi / 2))

# per-core: 8 pair blocks + tail (block, batch) or None (dummy)
CORE_PAIRS = [list(range(8 * c, 8 * c + 8)) for c in range(N_CORES)]
CORE_TAIL = [None] * 6 + [(64, 0), (64, 1)]


def _freqs_flat():
    f0 = np.fft.fftfreq(RES, d=1.0 / RES)          # kx: 0..63, -64..-1
    f1 = np.fft.rfftfreq(RES, d=1.0 / RES)         # ky: 0..64
    kx, ky = np.meshgrid(f0, f1, indexing="ij")
    return kx.reshape(-1).astype(np.float32), ky.reshape(-1).astype(np.float32)


def _build_program():
    import concourse.bacc as bacc
    import concourse.bass as bass
    import concourse.mybir as mybir
    from concourse.tile import TileContext

    f32 = mybir.dt.float32
    f32r = mybir.dt.float32r
    f16 = mybir.dt.float16
    bf16 = mybir.dt.bfloat16
    Alu = mybir.AluOpType
    Act = mybir.ActivationFunctionType

    nc = bacc.Bacc(None)
    # 9 lhsT blocks: 8 pair freq-blocks + tail freq-block
    kxyr = nc.dram_tensor("kxyr", [2, 9 * FPB], f32r, kind="ExternalInput")
    kxyn = nc.dram_tensor("kxyn", [2, 9 * FPB], f32r, kind="ExternalInput")
    # V columns: [b0 | b1 | tail-batch copy]
    vhi = nc.dram_tensor("vhi", [2, 3 * NV], f32r, kind="ExternalInput")
    vlo = nc.dram_tensor("vlo", [2, 3 * NV], f32r, kind="ExternalInput")
    vdw = nc.dram_tensor("vdw", [2, 3 * NV], f32r, kind="ExternalInput")
    fout = nc.dram_tensor("fout", [128, 34], f32, kind="ExternalOutput")

    def strided(ap, offset, outer_stride, outer_n, inner_n):
        return bass.AP(ap.tensor, ap.offset + offset,
                       [ap.ap[0], [outer_stride, outer_n], [1, inner_n]])

    with TileContext(nc) as tc:
        with (
            tc.tile_pool(name="const", bufs=1) as cpool,
            tc.tile_pool(name="work", bufs=WBUFS) as wpool,
            tc.tile_pool(name="ps_phi", bufs=PHIBUFS, space="PSUM") as ps_phi,
            tc.tile_pool(name="ps_rho", bufs=RHOBUFS, space="PSUM") as ps_rho,
        ):
            halfpi = cpool.tile([128, 1], f32)
            nc.gpsimd.memset(halfpi[:], _HALF_PI)
            kxy_sb = cpool.tile([2, 9 * FPB], f32r)
            kxyn_sb = cpool.tile([2, 9 * FPB], f32r)
            ones1 = cpool.tile([1, 128], bf16)
            nc.gpsimd.memset(ones1[:], 1.0)
            mrow = cpool.tile([1, NV], bf16)
            nc.gpsimd.memset(mrow[:], _MAGIC)
            nmrow = cpool.tile([1, NV], bf16)
            nc.gpsimd.memset(nmrow[:], -_MAGIC)
            vhi_sb = cpool.tile([2, 3 * NV], f32r)
            vlo_sb = cpool.tile([2, 3 * NV], f32r)
            vdw_sb = cpool.tile([2, 3 * NV], f32r)
            fout_sb = cpool.tile([128, 34], f32)
            nc.sync.dma_start(kxy_sb[:, :FPB], kxyr[:, :FPB])
            qs = [nc.sync, nc.scalar, nc.sync, nc.scalar]
            for t0 in range(3):
                cs = slice(t0 * NV, (t0 + 1) * NV)
                qs[0].dma_start(vhi_sb[:, cs], vhi[:, cs])
                qs[1].dma_start(vlo_sb[:, cs], vlo[:, cs])
                qs[2].dma_start(vdw_sb[:, cs], vdw[:, cs])
            nc.gpsimd.dma_start(kxy_sb[:, FPB:], kxyr[:, FPB:])
            nc.gpsimd.dma_start(kxyn_sb[:], kxyn[:])
            # activation-table warmup AFTER the DMA issues so the table
            # loads overlap the transfers instead of delaying them
            warm = cpool.tile([128, 1], f32)
            nc.scalar.activation(warm[:], halfpi[:], Act.Copy)
            nc.scalar.activation(warm[:], halfpi[:], Act.Sin)
            if PE_WARM:
                # ramp the PE p-state during the input DMA wait
                pe_warm = ps_rho.tile([128, 512], f32, tag="rho")
                for _ in range(PE_WARM):
                    nc.tensor.matmul(pe_warm[:], ones1[:], mrow[:],
                                     start=True, stop=True,
                                     skip_group_check=True)

            def emit_front(jblk, cols):
                """Matmuls + rounding chain + rphi + delta for one block."""
                nu = len(cols)
                W = 512 * nu
                lhsT = kxy_sb[:, jblk * FPB : (jblk + 1) * FPB]
                phi = ps_phi.tile([128, W], f32, tag="phi")
                rho = ps_rho.tile([128, W], f32, tag="rho")
                for _ in range(FILLER):
                    # PE keepalive: overwritten by the real rho matmul below
                    nc.tensor.matmul(rho[:, :512], ones1[:], mrow[:],
                                     start=True, stop=True,
                                     skip_group_check=True)
                for u, cb in enumerate(cols):
                    out = phi[:, 512 * u : 512 * u + 512]
                    nc.tensor.matmul(out, lhsT, vhi_sb[:, cb : cb + NV],
                                     start=True, stop=True,
                                     skip_group_check=True)
                    nc.tensor.matmul(out, lhsT, vlo_sb[:, cb : cb + NV],
                                     start=False, stop=True,
                                     skip_group_check=True)
                    nc.tensor.matmul(rho[:, 512 * u : 512 * u + 512],
                                     lhsT, vdw_sb[:, cb : cb + NV],
                                     start=True, stop=True,
                                     skip_group_check=True)
                # DVE: rphi before the bank is turned into -delta
                rphi16 = wpool.tile([128, W], f16, tag="rphi")
                with nc.allow_low_precision(reason="f16 pipeline"):
                    nc.vector.reciprocal(rphi16[:], phi[:])
                # PE in-bank range reduction:
                # bank = phi; += magic (rounds -> mphi); += -magic (= k,
                # exact); += -phi (hi+lo again, negated lhsT) -> k-phi = -delta
                lhsTn = kxyn_sb[:, jblk * FPB : (jblk + 1) * FPB]
                for u, cb in enumerate(cols):
                    out = phi[:, 512 * u : 512 * u + 512]
                    nc.tensor.matmul(out, ones1[:], mrow[:],
                                     start=False, stop=True,
                                     skip_group_check=True)
                    nc.tensor.matmul(out, ones1[:], nmrow[:],
                                     start=False, stop=True,
                                     skip_group_check=True)
                    nc.tensor.matmul(out, lhsTn, vhi_sb[:, cb : cb + NV],
                                     start=False, stop=True,
                                     skip_group_check=True)
                    nc.tensor.matmul(out, lhsTn, vlo_sb[:, cb : cb + NV],
                                     start=False, stop=True,
                                     skip_group_check=True)
                # DVE: rrw (needs only rho; overlaps the -delta chain)
                rrw = wpool.tile([128, 513 * nu], f16, tag="rrw")
                with nc.allow_low_precision(reason="f16 pipeline"):
                    nc.vector.reciprocal(strided(rrw[:], 1, 513, nu, 512),
                                         strided(rho[:], 0, 512, nu, 512))
                    for u in range(nu):
                        nc.vector.reciprocal(
                            rrw[:, 513 * u : 513 * u + 1],
                            rho[:, 512 * u + 511 : 512 * u + 512])
                return phi, rho, rphi16, rrw

            def emit_back(state, cols, nslots):
                phi_t, rho, rphi16, rrw = state
                nu = len(cols)
                W = 512 * nu
                phi = phi_t[:]
                # ACT: sh | ch packed (phi bank now holds -delta)
                shch = wpool.tile([128, 2 * W], f16, tag="shch")
                nc.scalar.activation(shch[:, :W], phi, Act.Sin, scale=_PI)
                nc.scalar.activation(shch[:, W:], phi, Act.Sin, scale=_PI,
                                     bias=halfpi[:])
                # Pool: a' = rrw[v-1]-rrw[v] ; A = a' * rphi
                tail_dve = TAIL_DVE and nu == 1
                veng = nc.vector if tail_dve else nc.gpsimd
                ap_t = wpool.tile([128, W], f16, tag="ap")
                (nc.vector if (tail_dve or AP_DVE) else nc.gpsimd).tensor_tensor(
                    strided(ap_t[:], 0, 512, nu, 512),
                    strided(rrw[:], 0, 513, nu, 512),
                    strided(rrw[:], 1, 513, nu, 512),
                    Alu.subtract)
                a_t = wpool.tile([128, W], f16, tag="a")
                veng.tensor_tensor(a_t[:], ap_t[:], rphi16[:], Alu.mult)
                # DVE: ash; Pool: t = [ash*sh | ash*ch]
                ash = wpool.tile([128, W], f16, tag="ash")
                if (nslots // 2) in ASH_POOL and not tail_dve:
                    nc.gpsimd.tensor_tensor(ash[:], a_t[:], shch[:, :W],
                                            Alu.mult)
                else:
                    nc.vector.tensor_tensor(ash[:], a_t[:], shch[:, :W],
                                            Alu.mult)
                t_t = wpool.tile([128, 2 * W], f16, tag="t")
                ash_rep = bass.AP(ash[:].tensor, ash[:].offset,
                                  [ash[:].ap[0], [0, 2], [1, W]])
                t_view = strided(t_t[:], 0, W, 2, W)
                sc_view = strided(shch[:], 0, W, 2, W)
                if (nslots // 2) in T_SPLIT:
                    nc.gpsimd.tensor_tensor(t_t[:, :W], ash[:], shch[:, :W],
                                            Alu.mult)
                    nc.gpsimd.tensor_tensor(t_t[:, W:], ash[:], shch[:, W:],
                                            Alu.mult)
                elif (nslots // 2) in T_DVE or tail_dve:
                    nc.vector.tensor_tensor(t_view, ash_rep, sc_view, Alu.mult)
                else:
                    nc.gpsimd.tensor_tensor(t_view, ash_rep, sc_view, Alu.mult)
                # reductions -> fout columns (DVE TSP 4x / ACT accum)
                trash = wpool.tile([128, 2 * W], f16, tag="trash")
                with nc.allow_low_precision(reason="fp32 accum"):
                    for u in range(nu):
                        for half in range(2):      # 0: re, 1: im
                            seg = slice(half * W + 512 * u,
                                        half * W + 512 * u + 512)
                            col = 2 * (nslots + u) + half
                            if (half, (nslots // 2)) in R_ACT:
                                nc.scalar.activation(
                                    trash[:, seg], t_t[:, seg], Act.Copy,
                                    accum_out=fout_sb[:, col : col + 1])
                            else:
                                nc.vector.tensor_scalar(
                                    trash[:, seg], t_t[:, seg], 1.0, 0.0,
                                    Alu.mult, Alu.add,
                                    accum_out=fout_sb[:, col : col + 1],
                                )

            if FIRST_SPLIT:
                plan = ([(0, [0], 0), (0, [NV], 1)]
                        + [(j, [0, NV], 2 * j) for j in range(1, 8)]
                        + [(8, [2 * NV], 16)])
            elif TAIL_POS is not None:
                pj = [(j, [0, NV], 2 * j) for j in range(8)]
                plan = (pj[:TAIL_POS] + [(8, [2 * NV], 16)] + pj[TAIL_POS:])
            else:
                plan = ([(j, [0, NV], 2 * j) for j in range(8)]
                        + [(8, [2 * NV], 16)])
            for j, cols, nslots in plan:
                st = emit_front(j, cols)
                emit_back(st, cols, nslots)
                if j == 3:
                    nc.sync.dma_start(fout[:, :16], fout_sb[:, :16])
                elif j == 7 and SPLIT_LAST_DMA:
                    nc.sync.dma_start(fout[:, 16:32], fout_sb[:, 16:32])
            if SPLIT_LAST_DMA:
                nc.sync.dma_start(fout[:, 32:], fout_sb[:, 32:])
            else:
                nc.sync.dma_start(fout[:, 16:], fout_sb[:, 16:])

    nc.compile()
    return nc


_PROGRAM = None


def _get_program():
    global _PROGRAM
    if _PROGRAM is None:
        _PROGRAM = _build_program()
    return _PROGRAM


def _ring_perm(Eb):
    """Vertex visiting order pi and per-position edge index for one batch."""
    a, bb = Eb[:, 0].astype(np.int64), Eb[:, 1].astype(np.int64)
    idx = np.arange(NV, dtype=np.int64)
    if np.array_equal(a, idx) and np.array_equal(bb, (idx + 1) % NV):
        return idx, idx                       # fast path: canonical ring
    succ = np.full(NV, -1, np.int64)
    eidx = np.full(NV, -1, np.int64)
    succ[a] = bb
    eidx[a] = idx
    pi = np.empty(NV, np.int64)
    cur = a[0]
    for j in range(NV):
        pi[j] = cur
        cur = succ[cur]
    if cur != pi[0] or len(np.unique(pi)) != NV:
        return None, None                     # not a single ring
    return pi, eidx[pi]


def _host_reference_bins(V0, V1, CD, kxf, kyf):
    """Exact (reference-semantics) recompute of a set of frequency bins."""
    om = np.stack([2 * np.pi * kxf, 2 * np.pi * kyf]).astype(np.float32)  # (2,nb)
    f = np.float32
    s0 = (V0.astype(f) @ om).astype(f)         # (NV, nb)
    s1 = (V1.astype(f) @ om).astype(f)
    r = f(s0 - s1)
    p = f(f(r * s0) * s1)
    with np.errstate(all="ignore"):
        q = np.where(p == 0, 0.0, 1.0 / p.astype(np.float64))
    u0 = 1 - np.cos(s0.astype(np.float64))
    u1 = 1 - np.cos(s1.astype(np.float64))
    tre = (u1 * s0 - u0 * s1) * q
    tim = (np.sin(s1.astype(np.float64)) * s0
           - np.sin(s0.astype(np.float64)) * s1) * q
    mask = (r == 0) | (s0 == 0) | (s1 == 0)
    tre = np.where(mask, 0.0, tre)
    tim = np.where(mask, 0.0, tim)
    w = -(RES ** 2) * CD.astype(np.float64)
    return w @ tre, w @ tim


def _host_fallback(V, E, D):
    """Reference-semantics host compute for non-ring edge lists."""
    kxf, kyf = _freqs_flat()
    Fre = np.zeros((B, NF), np.float64)
    Fim = np.zeros((B, NF), np.float64)
    for b in range(B):
        e0 = E[b, :, 0].astype(np.int64)
        e1 = E[b, :, 1].astype(np.int64)
        v0 = V[b, e0].astype(np.float32)
        v1 = V[b, e1].astype(np.float32)
        C = (v0[:, 0] * v1[:, 1]).astype(np.float32) - (
            v0[:, 1] * v1[:, 0]).astype(np.float32)
        CD = (C * D[b, :, 0].astype(np.float32)).astype(np.float32)
        for f0 in range(0, NF, 512):
            fs = slice(f0, f0 + 512)
            re, im = _host_reference_bins(v0, v1, CD, kxf[fs], kyf[fs])
            Fre[b, fs] = re
            Fim[b, fs] = im
        dc = CD.sum(dtype=np.float64) * (RES ** 2) / 2.0
        Fre[b, 0] = dc
        Fim[b, 0] = dc
    F = np.stack([Fre, Fim], axis=-1).astype(np.float32)
    return F.reshape(B, FX, FY, 1, 2)


def kernel(V, E, D):
    V = np.asarray(V)
    E = np.asarray(E)
    D = np.asarray(D)
    assert V.shape == (B, NV, 2) and E.shape == (B, NV, 2) and D.shape == (B, NV, 1)

    kxf, kyf = _freqs_flat()

    vhi = np.zeros((2, 3 * NV), np.float32)
    vlo = np.zeros((2, 3 * NV), np.float32)
    vdw = np.zeros((2, 3 * NV), np.float32)
    V0s, V1s, CDs = [], [], []
    ring_ok = True
    perms = []
    for b in range(B):
        pi, ei = _ring_perm(E[b])
        if pi is None:
            ring_ok = False
            break
        perms.append((pi, ei))
    if not ring_ok:
        return _host_fallback(V, E, D)
    for b in range(B):
        pi, ei = perms[b]
        v0 = V[b, pi].astype(np.float32)              # ring-ordered vertices
        v1 = np.roll(v0, -1, axis=0)
        C = (v0[:, 0] * v1[:, 1]).astype(np.float32) - (
            v0[:, 1] * v1[:, 0]).astype(np.float32)
        CD = (C * D[b, ei, 0].astype(np.float32)).astype(np.float32)
        V0s.append(v0); V1s.append(v1); CDs.append(CD)
        W = (np.float32(2.0 * (-(RES ** 2)) / (4 * np.pi ** 2) / (2 ** S_SCALE))
             * CD).astype(np.float32)
        Vd = (v0 - v1).astype(np.float32)
        with np.errstate(all="ignore"):
            Vw = (Vd / W[:, None]).astype(np.float32)
        badw = ~np.isfinite(Vw).all(axis=1)
        if badw.any():
            Vw[badw] = np.float32([0.5e30, 155.0 / 512.0 * 1e30])
        u = v0.view(np.uint32)
        hi = (u & np.uint32(0xFFFFF000)).view(np.float32)
        lo = (v0.astype(np.float64) - hi.astype(np.float64)).astype(np.float32)
        vhi[0, NV * b : NV * (b + 1)] = hi[:, 0]
        vhi[1, NV * b : NV * (b + 1)] = hi[:, 1]
        vlo[0, NV * b : NV * (b + 1)] = lo[:, 0]
        vlo[1, NV * b : NV * (b + 1)] = lo[:, 1]
        vdw[0, NV * b : NV * (b + 1)] = Vw[:, 0]
        vdw[1, NV * b : NV * (b + 1)] = Vw[:, 1]

    in_maps = []
    for c in range(N_CORES):
        kxy = np.zeros((2, 9 * FPB), np.float32)
        for j, fb in enumerate(CORE_PAIRS[c]):
            kxy[0, j * FPB : (j + 1) * FPB] = kxf[fb * FPB : (fb + 1) * FPB]
            kxy[1, j * FPB : (j + 1) * FPB] = kyf[fb * FPB : (fb + 1) * FPB]
        tail = CORE_TAIL[c]
        tfb, tb = (tail if tail is not None else (0, 0))
        kxy[0, 8 * FPB :] = kxf[tfb * FPB : (tfb + 1) * FPB]
        kxy[1, 8 * FPB :] = kyf[tfb * FPB : (tfb + 1) * FPB]
        # benign nonzero dummy freq for the DC bin (host overwrites it)
        kxy[:, (kxy[0] == 0.0) & (kxy[1] == 0.0)] = np.float32(
            [[0.5], [155.0 / 512.0]])
        vhi_c = vhi.copy(); vlo_c = vlo.copy(); vdw_c = vdw.copy()
        vhi_c[:, 2 * NV :] = vhi[:, tb * NV : (tb + 1) * NV]
        vlo_c[:, 2 * NV :] = vlo[:, tb * NV : (tb + 1) * NV]
        vdw_c[:, 2 * NV :] = vdw[:, tb * NV : (tb + 1) * NV]
        in_maps.append({
            "kxyr": kxy, "kxyn": -kxy, "vhi": vhi_c, "vlo": vlo_c,
            "vdw": vdw_c,
        })

    from concourse.bass_utils import run_bass_kernel_spmd

    nc = _get_program()
    res = run_bass_kernel_spmd(nc, in_maps, core_ids=list(range(N_CORES)))

    Fre = np.zeros((B, NF), np.float64)
    Fim = np.zeros((B, NF), np.float64)
    scale = float(2 ** S_SCALE)
    for c in range(N_CORES):
        part = res.results[c]["fout"].astype(np.float64)      # [128, 34]
        for j, fb in enumerate(CORE_PAIRS[c]):
            fr = slice(fb * FPB, (fb + 1) * FPB)
            for b in range(2):
                Fre[b, fr] = part[:, 2 * (2 * j + b)] * scale
                Fim[b, fr] = part[:, 2 * (2 * j + b) + 1] * -scale
        tail = CORE_TAIL[c]
        if tail is not None:
            tfb, tb = tail
            fr = slice(tfb * FPB, (tfb + 1) * FPB)
            Fre[tb, fr] = part[:, 32] * scale
            Fim[tb, fr] = part[:, 33] * -scale

    # host patch: recompute non-finite bins exactly
    for b in range(B):
        bad = ~(np.isfinite(Fre[b]) & np.isfinite(Fim[b]))
        bad[0] = False
        if bad.any():
            fi = np.where(bad)[0]
            re, im = _host_reference_bins(V0s[b], V1s[b], CDs[b],
                                          kxf[fi], kyf[fi])
            Fre[b, fi] = re
            Fim[b, fi] = im
        dc = CDs[b].sum(dtype=np.float64) * (RES ** 2) / 2.0
        Fre[b, 0] = dc
        Fim[b, 0] = dc

    F = np.stack([Fre, Fim], axis=-1).astype(np.float32)
    return F.reshape(B, FX, FY, 1, 2)


if __name__ == "__main__":
    rng = np.random.default_rng(0)
    Vt = rng.random((B, NV, 2), np.float32)
    idx = np.arange(NV, dtype=np.int32)
    Et = np.broadcast_to(
        np.stack([idx, (idx + 1) % NV], -1)[None], (B, NV, 2)
    ).astype(np.int32)
    Dt = np.ones((B, NV, 1), np.float32)
    out = kernel(V=Vt, E=Et, D=Dt)
    print(out.shape, out.dtype, np.abs(out).max())
